# revision 32
# baseline (speedup 1.0000x reference)
"""InternLM3 self-attention (prefill, GQA, RoPE) on 8 Trainium2 cores.

Tensor-parallel over heads: core r owns q heads 4r..4r+3 and kv head r
(wqkv column shards, wo row shards).  Each core computes its partial
output projection; the 8 partials are summed on the host.

Single fused pipeline per 512-token chunk t:
  QKV passes (one PSUM bank each) -> RoPE (q,k) -> v transpose ->
  causal attention group g=t -> softmax normalization -> wo matmul +
  bf16 store.  Emitted in one Tile scope so the scheduler overlaps
  chunk t+1's projection with chunk t's attention/output, keeping the
  PE warm (HAM at 8/8).

Everything is bf16 except q/k (f32r, softmax input precision) and
f32 PSUM/softmax internals.  Layout is fully transposed on-chip
(qkv^T = wqkv^T @ hidden^T) so scores^T = k^T-chunks @ q^T feed the
PV matmul with zero transposes; only v needs 16 tiny PE transposes.

Causal masking is a post-exp 0/1 bf16 multiply on the four diagonal
blocks; the softmax denominator accumulates on the PE into a [4, 512]
tile (one selector matmul per head) and uses the fast DVE reciprocal.
"""

import numpy as np
import ml_dtypes

import concourse.bass as bass
import concourse.bacc as bacc
import concourse.mybir as mybir
import concourse.tile as tile
from concourse.bass_utils import run_bass_kernel_spmd

T = 2048
H = 4096
NH = 32
NKV = 8
HD = 128
HALF = HD // 2
BASE = 1000000.0
NCORES = 8
QH = NH // NCORES            # 4 q heads per core
QCOLS = QH * HD              # 512
SH_COLS = QCOLS + 2 * HD     # 768 wqkv cols per core

P = 128
TC = 512                     # token chunk
NT = T // TC                 # 4
NHC = H // P                 # 32 contraction chunks
NKC = T // P                 # 16 k chunks of 128
SCALE = HD ** -0.5

f32 = mybir.dt.float32
f32r = mybir.dt.float32r
bf16 = mybir.dt.bfloat16
BF = ml_dtypes.bfloat16

_COMPILED = None
_LAST_IN_MAPS = None


def _build():
    nc = bacc.Bacc("TRN2", target_bir_lowering=False, debug=False,
                   num_devices=NCORES)

    # hidden pre-tiled host-side: [t, g, p, h, n] so each (t, g) hid
    # group load is one fully-contiguous DMA (4 KB per partition line)
    hid5 = nc.dram_tensor("hid5", [NT, 8, P, 4, TC], bf16,
                          kind="ExternalInput").ap()
    # wqkv shard pre-transposed host-side: [c-chunk, p, h, col] so one
    # column-chunk = one DMA with 8 KB contiguous per partition
    wqd = nc.dram_tensor("wqd", [6, P, NHC, P], bf16,
                         kind="ExternalInput").ap()
    wo_s = nc.dram_tensor("wo_s", [QCOLS, H], bf16,
                          kind="ExternalInput").ap()
    cosT = nc.dram_tensor("cosT", [P, T], f32, kind="ExternalInput").ap()
    sinrT = nc.dram_tensor("sinrT", [P, T], f32, kind="ExternalInput").ap()
    m01 = nc.dram_tensor("m01", [P, QH, TC], bf16,
                         kind="ExternalInput").ap()
    rperm = nc.dram_tensor("rperm", [P, P], bf16, kind="ExternalInput").ap()
    identb = nc.dram_tensor("identb", [P, P], bf16,
                            kind="ExternalInput").ap()
    selc = nc.dram_tensor("selc", [P, QH, QH], bf16,
                          kind="ExternalInput").ap()
    selr = nc.dram_tensor("selr", [QH, QH, P], bf16,
                          kind="ExternalInput").ap()
    part = nc.dram_tensor("part", [T, H], bf16, kind="ExternalOutput").ap()

    EXP = mybir.ActivationFunctionType.Exp
    MUL = mybir.AluOpType.mult
    ADD = mybir.AluOpType.add

    with tile.TileContext(nc) as tc:
        with tc.tile_pool(name="res", bufs=1) as res, \
             tc.tile_pool(name="hidp", bufs=10) as hidp, \
             tc.tile_pool(name="sb", bufs=2) as sb, \
             tc.tile_pool(name="accp", bufs=2, space="PSUM") as accp, \
             tc.tile_pool(name="stp", bufs=3, space="PSUM") as stp, \
             tc.tile_pool(name="pvp", bufs=2, space="PSUM") as pvp, \
             tc.tile_pool(name="smp", bufs=1, space="PSUM") as smp:

            # ---------------- resident SBUF ----------------
            wq = res.tile([P, 6, NHC, P], bf16)         # 48 KB
            wo_r = res.tile([P, QH, H], bf16)           # 32 KB
            kT = res.tile([P, T], bf16)                 # roped k^T, 4 KB
            vnat = res.tile([P, T], bf16)               # v natural, 4 KB
            ct = res.tile([P, T], f32)
            srt = res.tile([P, T], f32)
            mt = res.tile([P, QH, TC], bf16)
            rp = res.tile([P, P], bf16)
            idb = res.tile([P, P], bf16)
            slc = res.tile([P, QH, QH], bf16)
            slr = res.tile([QH, QH, P], bf16)

            # startup order tuned so pass c=0 (wq chunk 0 + all hid
            # groups of chunk 0) is fed within ~15us across 3 queues
            QS, QA, QG = nc.sync, nc.scalar, nc.gpsimd

            def load_hid_group(t, g, eng):
                # one DMA: 4 h-chunks of hidden for token chunk t
                ht = hidp.tile([P, 4, TC], bf16, tag="ht",
                               name=f"ht_{t}_{g}")
                eng.dma_start(ht[:], hid5[t, g, :, :, :])
                return ht

            hts0 = [None] * 8
            # fan the startup-critical loads over 4 DMA issuers, most
            # urgent first on each queue
            nc.sync.dma_start(wq[:, 0, :, :], wqd[0, :, :, :])
            hts0[0] = load_hid_group(0, 0, QA)
            hts0[1] = load_hid_group(0, 1, QG)
            hts0[2] = load_hid_group(0, 2, QS)
            nc.scalar.dma_start(wq[:, 1, :, :], wqd[1, :, :, :])
            nc.gpsimd.dma_start(wq[:, 2, :, :], wqd[2, :, :, :])
            hts0[3] = load_hid_group(0, 3, QS)
            hts0[4] = load_hid_group(0, 4, QA)
            hts0[5] = load_hid_group(0, 5, QG)
            nc.sync.dma_start(ct[:], cosT[:])
            nc.scalar.dma_start(wq[:, 3, :, :], wqd[3, :, :, :])
            nc.gpsimd.dma_start(wq[:, 4, :, :], wqd[4, :, :, :])
            hts0[6] = load_hid_group(0, 6, QA)
            hts0[7] = load_hid_group(0, 7, QG)
            nc.sync.dma_start(srt[:], sinrT[:])
            nc.scalar.dma_start(wq[:, 5, :, :], wqd[5, :, :, :])
            nc.sync.dma_start(mt[:], m01[:])
            nc.sync.dma_start(rp[:], rperm[:])
            nc.sync.dma_start(idb[:], identb[:])
            nc.sync.dma_start(slc[:], selc[:])
            nc.sync.dma_start(slr[:], selr[:])
            # wo by head-chunks (only needed by WO, ~70us in)
            for hc in range(QH):
                nc.sync.dma_start(
                    wo_r[:, hc, :], wo_s[hc * P:(hc + 1) * P, :])

            # warm up the PE clock gate (HAM) during the DMA preamble:
            # dependency-free matmuls on a memset tile bridge the ~13us
            # until the first real operands land, so stage 0 runs at
            # 2.4 GHz instead of 1.2
            wu = sb.tile([P, TC], bf16, tag="wu", bufs=1)
            nc.vector.memset(wu[:], 0)
            wu_ps = stp.tile([P, TC], f32, tag="st", name="wu_ps")
            for i in range(45):
                nc.tensor.matmul(wu_ps[:], wu[:, :P], wu[:],
                                 start=True, stop=True)

            for t in range(NT):
                tsl = slice(t * TC, (t + 1) * TC)

                # hid group tiles for chunk t.  Stage 0 fans across all
                # queues (fresh slots, waitless).  Later stages go on
                # gpsimd ONLY: their DMAs wait on hidp slot recycling,
                # and a parked DMA blocks every instruction behind it on
                # the issuing engine's queue -- gpsimd has nothing else.
                if t == 0:
                    htg = hts0
                elif t == 1:
                    # first two groups reuse nothing yet -> can ride
                    # scalar/sync without parking them
                    htg = [load_hid_group(t, 0, QA),
                           load_hid_group(t, 1, QS)] + \
                          [load_hid_group(t, g, QG) for g in range(2, 8)]
                else:
                    htg = [load_hid_group(t, g, QG) for g in range(8)]
                hts = [htg[h // 4][:, h % 4, :] for h in range(NHC)]

                qTg = sb.tile([P, QH, TC], bf16, tag="qTg",
                              name=f"qTg_{t}")

                # ---------- QKV projection passes ----------
                def qkv_pass(c):
                    qps = accp.tile([P, TC], f32, tag="acc",
                                    name=f"qps_{t}_{c}")
                    for h in range(NHC):
                        nc.tensor.matmul(
                            qps[:], wq[:, c, h, :], hts[h],
                            start=(h == 0), stop=(h == NHC - 1))
                    if c < 5:
                        # RoPE: dest = x*cos + rot64(x*sinrot)
                        dest = qTg[:, c, :] if c < QH else kT[:, tsl]
                        acos = sb.tile([P, TC], f32, tag="acos",
                                       name=f"acos_{t}_{c}")
                        nc.vector.tensor_tensor(acos[:], qps[:],
                                                ct[:, tsl], MUL)
                        bsb = sb.tile([P, TC], bf16, tag="bsb",
                                      name=f"bsb_{t}_{c}")
                        nc.vector.tensor_tensor(bsb[:], qps[:],
                                                srt[:, tsl], MUL)
                        rps = stp.tile([P, TC], f32, tag="st",
                                       name=f"rps_{t}_{c}")
                        nc.tensor.matmul(rps[:], rp[:], bsb[:],
                                         start=True, stop=True)
                        nc.vector.tensor_tensor(dest, acos[:], rps[:],
                                                ADD)
                    else:
                        # v: evac + 4 transposes into natural layout
                        vsb = sb.tile([P, TC], bf16, tag="vsb",
                                      name=f"vsb_{t}")
                        nc.vector.tensor_copy(vsb[:], qps[:])
                        for j in range(4):
                            tp = stp.tile([P, P], bf16, tag="st",
                                          name=f"tp_{t}_{j}")
                            nc.tensor.transpose(
                                tp[:], vsb[:, j * P:(j + 1) * P], idb[:])
                            kc = 4 * t + j
                            nc.vector.tensor_copy(
                                vnat[:, kc * P:(kc + 1) * P], tp[:])

                # ---------- attention group g = t ----------
                kmax = 4 * (t + 1)
                d4 = smp.tile([QH, TC], f32, tag="d4", name=f"d4_{t}")

                def att_block(head, pv, es, kc):
                    # diagonal block j: only q >= 128*j is unmasked,
                    # so shrink the moving operand to N = 512-128*j
                    j = kc - 4 * t
                    off = max(0, j) * P
                    n = TC - off
                    qsl = slice(off, TC)
                    st = stp.tile([P, TC], f32, tag="st",
                                  name=f"st_{t}_{head}_{kc}")
                    nc.tensor.matmul(
                        st[:, :n], kT[:, kc * P:(kc + 1) * P],
                        qTg[:, head, qsl], start=True, stop=True)
                    e = sb.tile([P, TC], bf16, tag="e", bufs=6,
                                name=f"e_{t}_{head}_{kc}")
                    nc.scalar.activation(e[:, :n], st[:, :n], EXP,
                                         scale=SCALE)
                    if j >= 0:
                        nc.vector.tensor_tensor(e[:, :n], e[:, :n],
                                                mt[:, j, qsl], MUL)
                    if kc == 0:
                        nc.vector.tensor_copy(es[:], e[:])
                    else:
                        nc.vector.tensor_tensor(es[:, qsl], es[:, qsl],
                                                e[:, :n], ADD)
                    nc.tensor.matmul(
                        pv[:, qsl], vnat[:, kc * P:(kc + 1) * P],
                        e[:, :n], start=(kc == 0),
                        stop=(kc == kmax - 1))

                def att_fin(head, pv, es):
                    pvsb = sb.tile([P, TC], bf16, tag="pvsb", bufs=5,
                                   name=f"pvsb_{t}_{head}")
                    nc.vector.tensor_copy(pvsb[:], pv[:])
                    pvs.append(pvsb)
                    # denominator: partition `head` of d4 += colsum(es)
                    nc.tensor.matmul(d4[:], slc[:, head, :], es[:],
                                     start=(head == 0),
                                     stop=(head == QH - 1))

                # q passes + rope first, then the off-diagonal attention
                # of heads 0-1 overlaps the k/v passes; heads are
                # processed in block-interleaved pairs so the PE never
                # trails a single st->exp->pv chain
                def att_pair(h0, kc_range, state):
                    for kc in kc_range:
                        for head in (h0, h0 + 1):
                            att_block(head, *state[head - h0], kc)

                for c in range(4):
                    qkv_pass(c)
                st01 = []
                for head in range(2):
                    pv = pvp.tile([P, TC], f32, tag="pv",
                                  name=f"pv_{t}_{head}")
                    es = sb.tile([P, TC], bf16, tag="es", bufs=3,
                                 name=f"es_{t}_{head}")
                    st01.append((pv, es))
                att_pair(0, range(4 * t), st01)
                qkv_pass(4)
                qkv_pass(5)
                pvs = []
                att_pair(0, range(4 * t, kmax), st01)
                for head in range(2):
                    att_fin(head, *st01[head])
                st23 = []
                for head in range(2, QH):
                    pv = pvp.tile([P, TC], f32, tag="pv",
                                  name=f"pv_{t}_{head}")
                    es = sb.tile([P, TC], bf16, tag="es", bufs=3,
                                 name=f"es_{t}_{head}")
                    st23.append((pv, es))
                att_pair(2, range(kmax), st23)
                for head in range(2, QH):
                    att_fin(head, *st23[head - 2])

                # softmax normalization for the whole group
                rd = sb.tile([QH, TC], f32, tag="rd", name=f"rd_{t}")
                nc.vector.reciprocal_approx_fast(rd[:], d4[:])
                rdr = sb.tile([QH, TC], bf16, tag="rdr", name=f"rdr_{t}")
                nc.vector.tensor_copy(rdr[:], rd[:])
                atg = sb.tile([P, QH, TC], bf16, tag="atg",
                              name=f"atg_{t}")
                for head in range(QH):
                    rb = stp.tile([P, TC], f32, tag="st",
                                  name=f"rb_{t}_{head}")
                    nc.tensor.matmul(rb[:], slr[:, head, :], rdr[:],
                                     start=True, stop=True)
                    rbs = sb.tile([P, TC], bf16, tag="rbs",
                                  name=f"rbs_{t}_{head}")
                    nc.scalar.copy(rbs[:], rb[:])
                    nc.vector.tensor_tensor(atg[:, head, :], pvs[head][:],
                                            rbs[:], MUL)

                # ---------- output projection for group t ----------
                for tq in range(4):
                    tcn = 4 * t + tq
                    for half in range(2):
                        osb = sb.tile([P, H // 2], bf16, tag="osb",
                                      name=f"osb_{tcn}_{half}")
                        for oi in range(4):
                            oc = half * 4 + oi
                            ops_t = accp.tile([P, TC], f32, tag="acc",
                                              name=f"o_{tcn}_{oc}")
                            for hc in range(QH):
                                nc.tensor.matmul(
                                    ops_t[:],
                                    atg[:, hc, tq * P:(tq + 1) * P],
                                    wo_r[:, hc, oc * TC:(oc + 1) * TC],
                                    start=(hc == 0), stop=(hc == QH - 1))
                            osl = osb[:, oi * TC:(oi + 1) * TC]
                            if oc % 2 == 0:
                                nc.vector.tensor_copy(osl, ops_t[:])
                            else:
                                nc.scalar.copy(osl, ops_t[:])
                        nc.sync.dma_start(
                            part[tcn * P:(tcn + 1) * P,
                                 half * (H // 2):(half + 1) * (H // 2)],
                            osb[:])

    nc.compile()
    return nc


def _tables(positions):
    pos = positions.astype(np.float64)
    inv_freq = 1.0 / (BASE ** (np.arange(HALF, dtype=np.float64) / HALF))
    freqs = pos[:, None] * inv_freq[None, :]          # [T, 64]
    cos = np.cos(freqs)
    sin = np.sin(freqs)
    cosT = np.concatenate([cos, cos], axis=1).T       # [128, T]
    sinT = np.concatenate([-sin, sin], axis=1).T      # sign folded
    sinrT = np.roll(sinT, -HALF, axis=0)              # pre-rotated by 64
    return cosT.astype(np.float32), sinrT.astype(np.float32)


def kernel(positions, hidden_states, wqkv, wo):
    global _COMPILED, _LAST_IN_MAPS
    if _COMPILED is None:
        _COMPILED = _build()
    nc = _COMPILED

    positions = np.asarray(positions)
    hidden_states = np.asarray(hidden_states)
    wqkv = np.asarray(wqkv)
    wo = np.asarray(wo)

    cosT, sinrT = _tables(positions)
    # [t, g, p, h, n] tiling of hidden^T (fully contiguous group DMAs)
    hid5 = np.ascontiguousarray(
        hidden_states.T.reshape(8, 4, P, NT, TC).transpose(
            3, 0, 2, 1, 4)).astype(BF)

    # 0/1 causal masks for the 4 diagonal sub-blocks, ^T layout [k, q]
    kl = np.arange(P)[:, None]
    ql = np.arange(TC)[None, :]
    m01 = np.stack(
        [np.where(P * j + kl <= ql, 1.0, 0.0) for j in range(4)],
        axis=1).astype(BF)                            # [128, 4, 512]

    rperm = np.zeros((P, P), dtype=np.float32)
    for m in range(P):
        rperm[(m + HALF) % P, m] = 1.0                # out[m]=x[(m+64)%128]
    rperm = rperm.astype(BF)
    identb = np.eye(P, dtype=BF)
    selc = np.zeros((P, QH, QH), dtype=BF)
    selr = np.zeros((QH, QH, P), dtype=BF)
    for h in range(QH):
        selc[:, h, h] = 1.0
        selr[h, h, :] = 1.0

    in_maps = []
    for r in range(NCORES):
        qc = slice(r * QCOLS, (r + 1) * QCOLS)
        kc = slice(NH * HD + r * HD, NH * HD + (r + 1) * HD)
        vc = slice((NH + NKV) * HD + r * HD, (NH + NKV) * HD + (r + 1) * HD)
        wqkv_s = np.concatenate([wqkv[:, qc], wqkv[:, kc], wqkv[:, vc]],
                                axis=1)
        # [c-chunk, p, h, col] so each c-chunk is contiguous per partition
        wqd = np.ascontiguousarray(
            wqkv_s.reshape(NHC, P, 6, P).transpose(2, 1, 0, 3)).astype(BF)
        wo_s = np.ascontiguousarray(wo[qc, :]).astype(BF)
        in_maps.append({
            "hid5": hid5, "wqd": wqd, "wo_s": wo_s,
            "cosT": cosT, "sinrT": sinrT, "m01": m01, "rperm": rperm,
            "identb": identb, "selc": selc, "selr": selr,
        })

    _LAST_IN_MAPS = in_maps
    res = run_bass_kernel_spmd(nc, in_maps, list(range(NCORES)))
    out = res.results[0]["part"].astype(np.float64)
    for r in range(1, NCORES):
        out += res.results[r]["part"].astype(np.float64)
    return out.astype(np.float32)
